# revision 29
# baseline (speedup 1.0000x reference)
"""Trainium2 Bass kernel for nn_AttentionAllocationSystem (gnn_message_passing).

Reference computation (per batch b of 64):
  QKV net over 128 defenders (6 -> 32 -> 32 -> q/k/v[16]), softmax attention
  over defenders, ctx[128,16]; then a pairwise MLP over all (defender,
  attacker) pairs: concat(def[6], ctx[16], att[6]) -> 64 -> 64 -> 32 -> 1,
  output [64, 128, 128].

Sharding: pure data parallel, batch 64 split as 8 per core across 8 cores.

Kernel strategy (per core, 8 batches):
 * Layer-1 rank split: x @ eW1 = enriched @ eW1[:22] + att @ eW1[22:], so
   per batch compute U[64,128] (per defender) and V[64,128] (per attacker)
   once; h1(d,a) = relu(U[:,d] + V[:,a]) comes from tensor_scalar(add,max)
   ops with U columns as the per-partition scalar operand.
 * 2-way feature stacking: column (dp, a) holds points (d=dp, a) in rows
   0:64 and (d=dp+64, a) in rows 64:128 -> every op uses 128 partitions,
   L2/L3 weights are block-diagonal.
 * L4 accumulation: 16 shifted [64,32] weight variants accumulate all 16
   chunks of a batch into one [32,512] PSUM tile whose rows map linearly to
   the output (row p = defenders 4p..4p+3), so the final store is a single
   contiguous DMA.
"""

import os
import sys

import numpy as np

sys.path.insert(0, "/opt/trn_rl_repo")

import concourse.bass as bass  # noqa: E402
import concourse.tile as tile  # noqa: E402
from concourse import bacc, mybir  # noqa: E402
from concourse.masks import make_identity  # noqa: E402

F32 = mybir.dt.float32
F32R = mybir.dt.float32r  # relaxed-precision fp32: full-rate on the PE
AF = mybir.ActivationFunctionType
ALU = mybir.AluOpType

N_CORES = 8
B, ND, NA = 64, 128, 128
BPC = B // N_CORES  # batches per core
NCHUNK = 16         # chunks per batch; each chunk = 4 dp groups x 128 attackers

# h1 engine assignment per m in 0..3: "v"=vector, "g"=gpsimd, "s"=scalar
H1_ENGINES = os.environ.get("H1_ENGINES", "vvgg")
# main-loop matmul operand dtype: f32r (full-rate PE) or f32 (4 cyc/row)
MM_F32R = bool(int(os.environ.get("MM_F32R", "1")))
FMM = F32R if MM_F32R else F32
# repeat the whole computation R times inside one NEFF (timing only)
REPEAT = int(os.environ.get("KERNEL_REPEAT", "1"))


def _weight_tensors(d):
    """Host-side preprocessing of the reference weights into kernel layout."""
    w = {}
    w["fW1"] = d["fW1"]                               # [6, 32]
    w["fb1c"] = d["fb1"].reshape(32, 1)
    w["fW2"] = d["fW2"]                               # [32, 32]
    w["fb2c"] = d["fb2"].reshape(32, 1)
    w["qWs"] = d["qW"] * 0.25                         # [32, 16] (1/sqrt(16) folded)
    w["qbsc"] = (d["qb"] * 0.25).reshape(16, 1)
    w["kW"] = d["kW"]
    w["kbc"] = d["kb"].reshape(16, 1)
    w["vW"] = d["vW"]
    w["vbc"] = d["vb"].reshape(16, 1)
    w["eW1d"] = d["eW1"][:6].copy()                   # [6, 64]  (defender rows)
    w["eW1c"] = d["eW1"][6:22].copy()                 # [16, 64] (ctx rows)
    w["eW1b"] = d["eW1"][22:].copy()                  # [6, 64]  (attacker rows)
    w["b1sc"] = np.concatenate([d["eb1"], d["eb1"]]).reshape(128, 1)
    W2s = np.zeros((128, 128), np.float32)
    W2s[:64, :64] = d["eW2"]
    W2s[64:, 64:] = d["eW2"]
    w["W2s"] = W2s
    w["b2sc"] = np.concatenate([d["eb2"], d["eb2"]]).reshape(128, 1)
    # W3pair[:, h, :]: L3 stationary for pair-half h, M=128 at base 0 so the
    # two halves accumulate into one [128,512] PSUM tile (f32r requires
    # base-0 outputs). Half h fills output rows 64h..64h+64; rest is zero.
    W3pair = np.zeros((128, 2, 128), np.float32)
    for h in range(2):
        W3pair[0:64, h, 64 * h:64 * h + 32] = d["eW3"]
        W3pair[64:128, h, 64 * h + 32:64 * h + 64] = d["eW3"]
    w["W3pair"] = W3pair
    w["b3sc2"] = np.concatenate([d["eb3"]] * 4).reshape(128, 1)
    # W4pair[:, j, :]: chunk-pair j's [128, 32] stationary for the fused L4
    # over two chunks (2c, 2c+1) stacked in partitions. Chunk c's eW4 copies
    # land in columns c (rows 0:32, d=dp points) and c+16 (rows 32:64,
    # d=dp+64 points), so PSUM row p = c + 16 r maps to defenders d=4p..4p+3.
    W4pair = np.zeros((128, NCHUNK // 2, 32), np.float32)
    for j in range(NCHUNK // 2):
        c0, c1 = 2 * j, 2 * j + 1
        W4pair[0:32, j, c0] = d["eW4"][:, 0]
        W4pair[32:64, j, c0 + 16] = d["eW4"][:, 0]
        W4pair[64:96, j, c1] = d["eW4"][:, 0]
        W4pair[96:128, j, c1 + 16] = d["eW4"][:, 0]
    w["W4pair"] = W4pair
    w["b4c"] = np.full((32, 1), float(d["eb4"][0]), np.float32)
    return {k: np.ascontiguousarray(v, np.float32) for k, v in w.items()}


def build_nc(repeat=None):
    global REPEAT
    if repeat is not None:
        REPEAT = repeat
    nc = bacc.Bacc()

    def_in = nc.dram_tensor("def_feat", [BPC, ND, 6], F32, kind="ExternalInput")
    att_in = nc.dram_tensor("att_feat", [BPC, NA, 6], F32, kind="ExternalInput")
    wshapes = {
        "fW1": [6, 32], "fb1c": [32, 1], "fW2": [32, 32], "fb2c": [32, 1],
        "qWs": [32, 16], "qbsc": [16, 1], "kW": [32, 16], "kbc": [16, 1],
        "vW": [32, 16], "vbc": [16, 1], "eW1d": [6, 64], "eW1c": [16, 64],
        "eW1b": [6, 64],
        "b1sc": [128, 1], "W2s": [128, 128], "b2sc": [128, 1],
        "W3pair": [128, 2, 128], "b3sc2": [128, 1],
        "W4pair": [128, NCHUNK // 2, 32], "b4c": [32, 1],
    }
    mm_weights = {"W2s", "W3pair", "W4pair"}
    wdram = {k: nc.dram_tensor(k, s, FMM if k in mm_weights else F32,
                               kind="ExternalInput")
             for k, s in wshapes.items()}
    out_dram = nc.dram_tensor("out", [BPC, ND, NA], F32, kind="ExternalOutput")

    with tile.TileContext(nc) as tc:
        with (
            tc.tile_pool(name="const", bufs=1) as const,
            tc.tile_pool(name="wide", bufs=1) as wide,
            tc.tile_pool(name="uv", bufs=1) as uv,
            tc.tile_pool(name="pre", bufs=3) as pre,
            tc.tile_pool(name="h1p", bufs=3) as h1p,
            tc.tile_pool(name="h2p", bufs=3) as h2p,
            tc.tile_pool(name="h3p", bufs=3) as h3p,
            tc.tile_pool(name="outp", bufs=2) as outp,
            tc.tile_pool(name="preps", bufs=2, space="PSUM") as preps,
            tc.tile_pool(name="ps2p", bufs=2, space="PSUM") as ps2p,
            tc.tile_pool(name="ps3p", bufs=2, space="PSUM") as ps3p,
            tc.tile_pool(name="outps", bufs=2, space="PSUM") as outps,
        ):
            wsb = {}
            for k, s in wshapes.items():
                wsb[k] = const.tile(s, FMM if k in mm_weights else F32,
                                    tag=k, name=k)
                nc.sync.dma_start(out=wsb[k][:], in_=wdram[k][:])
            identity = const.tile([128, 128], F32, tag="identity")
            make_identity(nc, identity[:])
            def_all = const.tile([128, BPC, 6], F32, tag="def_all")
            att_all = const.tile([128, BPC, 6], F32, tag="att_all")
            nc.sync.dma_start(out=def_all[:],
                              in_=def_in.rearrange("b d f -> d b f"))
            nc.sync.dma_start(out=att_all[:],
                              in_=att_in.rearrange("b a f -> a b f"))
            # feature-major inputs via PE transpose: [feat, batch, pos]
            def_T_all = const.tile([6, BPC, 128], F32, tag="def_T_all")
            att_T_all = const.tile([6, BPC, 128], F32, tag="att_T_all")
            for b in range(BPC):
                ps_t1 = preps.tile([6, 128], F32, tag="preps", name="ps_t1")
                nc.tensor.transpose(ps_t1[:], def_all[:, b, :], identity[:])
                nc.vector.tensor_copy(def_T_all[:, b, :], ps_t1[:])
                ps_t2 = preps.tile([6, 128], F32, tag="preps", name="ps_t2")
                nc.tensor.transpose(ps_t2[:], att_all[:, b, :], identity[:])
                nc.vector.tensor_copy(att_T_all[:, b, :], ps_t2[:])

            for rep in range(REPEAT):
                # ---- batched QKV net over all 8 batches (N=512 x2 wide) ----
                defT_flat = def_T_all[:].rearrange("f b d -> f (b d)")
                h1a_all = wide.tile([32, BPC * 128], F32, tag="h1a_all")
                h2a_all = wide.tile([32, BPC * 128], F32, tag="h2a_all")
                qT_all = wide.tile([16, BPC * 128], F32, tag="qT_all")
                kT_all = wide.tile([16, BPC * 128], F32, tag="kT_all")
                for dst, mout, lhs, rhs, bias, func in (
                    (h1a_all, 32, "fW1", defT_flat, "fb1c", AF.Relu),
                    (h2a_all, 32, "fW2", h1a_all[:], "fb2c", AF.Relu),
                    (qT_all, 16, "qWs", h2a_all[:], "qbsc", AF.Identity),
                    (kT_all, 16, "kW", h2a_all[:], "kbc", AF.Identity),
                ):
                    for s in range(BPC * 128 // 512):
                        sl = slice(512 * s, 512 * (s + 1))
                        ps_ = ps3p.tile([mout, 512], F32, tag="ps3",
                                        name="ps_wide")
                        nc.tensor.matmul(ps_[:], wsb[lhs][:], rhs[:, sl],
                                         start=True, stop=True)
                        nc.scalar.activation(dst[:, sl], ps_[:], func,
                                             bias=wsb[bias][:])

                # ---- per-batch attention preambles (independent chains) ----
                U2s, V2s = [], []
                for b in range(BPC):
                    bs = slice(128 * b, 128 * (b + 1))
                    def_T = def_T_all[:, b, :]
                    att_T = att_T_all[:, b, :]

                    ps_v = preps.tile([128, 16], F32, tag="preps", name="ps_v")
                    nc.tensor.matmul(ps_v[:], h2a_all[:, bs], wsb["vW"][:],
                                     start=True, stop=True)
                    v0 = pre.tile([128, 16], F32, tag="v0")
                    nc.vector.tensor_copy(v0[:], ps_v[:])

                    ps_sc = preps.tile([128, 128], F32, tag="preps",
                                       name="ps_sc")
                    nc.tensor.matmul(ps_sc[:], qT_all[:, bs], kT_all[:, bs],
                                     start=True, stop=True)
                    rmax_neg = pre.tile([128, 1], F32, tag="rmax_neg")
                    nc.vector.tensor_reduce(rmax_neg[:], ps_sc[:],
                                            axis=mybir.AxisListType.X,
                                            op=ALU.max, negate=True)
                    e_sb = pre.tile([128, 128], F32, tag="e_sb")
                    rsum = pre.tile([128, 1], F32, tag="rsum")
                    nc.scalar.activation(e_sb[:], ps_sc[:], AF.Exp,
                                         bias=rmax_neg[:], accum_out=rsum[:])
                    rinv = pre.tile([128, 1], F32, tag="rinv")
                    nc.vector.reciprocal(rinv[:], rsum[:])
                    attn = pre.tile([128, 128], F32, tag="attn")
                    nc.vector.tensor_scalar_mul(attn[:], e_sb[:], rinv[:])

                    ps_at = preps.tile([128, 128], F32, tag="preps",
                                       name="ps_at")
                    nc.tensor.transpose(ps_at[:], attn[:], identity[:])
                    attn_T = pre.tile([128, 128], F32, tag="attn_T")
                    nc.vector.tensor_copy(attn_T[:], ps_at[:])

                    ps_ctx = preps.tile([16, 128], F32, tag="preps",
                                        name="ps_ctx")
                    nc.tensor.matmul(ps_ctx[:], v0[:], attn_T[:],
                                     start=True, stop=True)
                    ctx_T = pre.tile([16, 128], F32, tag="ctx_T")
                    nc.scalar.activation(ctx_T[:], ps_ctx[:], AF.Identity,
                                         bias=wsb["vbc"][:])

                    # U2[p, dp] = (eW1d.T @ def_T + eW1c.T @ ctx_T)[:, dp(+64)]
                    ps_u2 = preps.tile([128, 64], F32, tag="preps",
                                       name="ps_u2")
                    nc.tensor.matmul(ps_u2[0:64, :], wsb["eW1d"][:],
                                     def_T[:, 0:64], start=True, stop=False)
                    nc.tensor.matmul(ps_u2[0:64, :], wsb["eW1c"][:],
                                     ctx_T[:, 0:64], start=False, stop=True)
                    nc.tensor.matmul(ps_u2[64:128, :], wsb["eW1d"][:],
                                     def_T[:, 64:128], start=True, stop=False)
                    nc.tensor.matmul(ps_u2[64:128, :], wsb["eW1c"][:],
                                     ctx_T[:, 64:128], start=False, stop=True)
                    U2 = uv.tile([128, 64], F32, tag=f"U2_{b}", name="U2")
                    nc.vector.tensor_copy(U2[:], ps_u2[:])

                    ps_v2 = preps.tile([128, 128], F32, tag="preps",
                                       name="ps_v2")
                    nc.tensor.matmul(ps_v2[0:64, :], wsb["eW1b"][:], att_T,
                                     start=True, stop=True)
                    nc.tensor.matmul(ps_v2[64:128, :], wsb["eW1b"][:], att_T,
                                     start=True, stop=True)
                    V2 = uv.tile([128, 128], F32, tag=f"V2_{b}", name="V2")
                    nc.scalar.activation(V2[:], ps_v2[:], AF.Identity,
                                         bias=wsb["b1sc"][:])
                    U2s.append(U2)
                    V2s.append(V2)

                # ---- main loops ----
                for b in range(BPC):
                    _main_loop(nc, wsb, outp, outps, h1p, h2p, h3p,
                               ps2p, ps3p, U2s[b], V2s[b], out_dram, b)
    nc.finalize()
    return nc


def _main_loop(nc, wsb, outp, outps, h1p, h2p, h3p, ps2p, ps3p,
               U2, V2, out_dram, b):
    """Pairwise MLP over one batch: 8 chunk pairs of 1024 points each."""
    out_ps = outps.tile([32, 512], F32, tag="outps", name="out_ps")
    NPAIR = NCHUNK // 2
    for j in range(NPAIR):
        ps34 = ps3p.tile([128, 512], F32, tag="ps3", name="ps34")
        for half in range(2):
            c = 2 * j + half
            h1 = h1p.tile([128, 512], FMM, tag="h1", name="h1")
            for m in range(4):
                dp = 4 * c + m
                seg = h1[:, 128 * m:128 * (m + 1)]
                eng = H1_ENGINES[m]
                if eng == "s":
                    nc.scalar.activation(seg, V2[:], AF.Relu,
                                         bias=U2[:, dp:dp + 1])
                else:
                    e_ = nc.vector if eng == "v" else nc.gpsimd
                    e_.tensor_scalar(seg, V2[:], U2[:, dp:dp + 1],
                                     0.0, ALU.add, ALU.max)
            ps2 = ps2p.tile([128, 512], F32, tag="ps2", name="ps2")
            nc.tensor.matmul(ps2[:], wsb["W2s"][:], h1[:],
                             start=True, stop=True)
            h2 = h2p.tile([128, 512], FMM, tag="h2", name="h2")
            nc.scalar.activation(h2[:], ps2[:], AF.Relu, bias=wsb["b2sc"][:])
            # two chunks' L3 outputs stack into one [128,512] psum
            # (accumulating M=128 matmuls with half-zero weights)
            nc.tensor.matmul(ps34[:], wsb["W3pair"][:, half, :], h2[:],
                             start=(half == 0), stop=(half == 1))
        h34 = h3p.tile([128, 512], FMM, tag="h3", name="h34")
        nc.vector.tensor_scalar(h34[:], ps34[:], wsb["b3sc2"][:],
                                0.0, ALU.add, ALU.max)
        nc.tensor.matmul(out_ps[:], wsb["W4pair"][:, j, :], h34[:],
                         start=(j == 0), stop=(j == NPAIR - 1))

    out_sb = outp.tile([32, 512], F32, tag="out_sb", name="out_sb")
    nc.scalar.activation(out_sb[:], out_ps[:], AF.Identity, bias=wsb["b4c"][:])
    # row p holds defenders d = 4p..4p+3 -> contiguous store
    nc.sync.dma_start(
        out=out_dram[b].rearrange("(p m) a -> p (m a)", p=32),
        in_=out_sb[:])


_NC_CACHE = {}


def kernel(defender_features, attacker_features, **weights):
    """Full-input entry point: shards batch across 8 cores, returns [64,128,128]."""
    from concourse import bass2jax

    if "nc" not in _NC_CACHE:
        _NC_CACHE["nc"] = build_nc()
    nc = _NC_CACHE["nc"]

    w = _weight_tensors(weights)
    df = np.ascontiguousarray(defender_features, np.float32)
    af = np.ascontiguousarray(attacker_features, np.float32)
    in_maps = []
    for i in range(N_CORES):
        m = {"def_feat": df[i * BPC:(i + 1) * BPC],
             "att_feat": af[i * BPC:(i + 1) * BPC]}
        m.update(w)
        in_maps.append(m)

    results = bass2jax.run_bass_via_pjrt(nc, in_maps, n_cores=N_CORES)
    return np.concatenate([results[i]["out"] for i in range(N_CORES)], axis=0)


if __name__ == "__main__":
    build_nc()
    print("build ok")


# revision 35
# speedup vs baseline: 4.1015x; 4.1015x over previous
"""Trainium2 Bass kernel for nn_AttentionAllocationSystem (gnn_message_passing).

Reference computation (per batch b of 64):
  QKV net over 128 defenders (6 -> 32 -> 32 -> q/k/v[16]), softmax attention
  over defenders, ctx[128,16]; then a pairwise MLP over all (defender,
  attacker) pairs: concat(def[6], ctx[16], att[6]) -> 64 -> 64 -> 32 -> 1,
  output [64, 128, 128].

Sharding: pure data parallel, batch 64 split as 8 per core across 8 cores.

Kernel strategy (per core, 8 batches):
 * Layer-1 rank split: x @ eW1 = enriched @ eW1[:22] + att @ eW1[22:], so
   per batch compute U[64,128] (per defender) and V[64,128] (per attacker)
   once; h1(d,a) = relu(U[:,d] + V[:,a]) comes from tensor_scalar(add,max)
   ops with U columns as the per-partition scalar operand.
 * 2-way feature stacking: column (dp, a) holds points (d=dp, a) in rows
   0:64 and (d=dp+64, a) in rows 64:128 -> every op uses 128 partitions,
   L2/L3 weights are block-diagonal.
 * L4 accumulation: 16 shifted [64,32] weight variants accumulate all 16
   chunks of a batch into one [32,512] PSUM tile whose rows map linearly to
   the output (row p = defenders 4p..4p+3), so the final store is a single
   contiguous DMA.
"""

import os
import sys

import numpy as np

sys.path.insert(0, "/opt/trn_rl_repo")

import concourse.bass as bass  # noqa: E402
import concourse.tile as tile  # noqa: E402
from concourse import bacc, mybir  # noqa: E402
from concourse.masks import make_identity  # noqa: E402

F32 = mybir.dt.float32
F32R = mybir.dt.float32r  # relaxed-precision fp32: full-rate on the PE
BF16 = mybir.dt.bfloat16
AF = mybir.ActivationFunctionType
ALU = mybir.AluOpType

N_CORES = 8
B, ND, NA = 64, 128, 128
BPC = B // N_CORES  # batches per core
NCHUNK = 16         # chunks per batch; each chunk = 4 dp groups x 128 attackers

# h1 engine assignment per m in 0..3: "v"=vector, "g"=gpsimd, "s"=scalar
H1_ENGINES = os.environ.get("H1_ENGINES", "vvgg")
# main-loop matmul operand dtype: f32r (full-rate PE) or f32 (4 cyc/row)
MM_F32R = bool(int(os.environ.get("MM_F32R", "1")))
FMM = F32R if MM_F32R else F32
# bf16 main-loop activations/weights: halves DVE evict cycles (4x/2x modes)
EVICT16 = bool(int(os.environ.get("EVICT16", "0")))
if EVICT16:
    FMM = BF16
# repeat the whole computation R times inside one NEFF (timing only)
REPEAT = int(os.environ.get("KERNEL_REPEAT", "1"))


def _weight_tensors(d):
    """Host-side preprocessing of the reference weights into kernel layout."""
    w = {}
    w["fW1"] = d["fW1"]                               # [6, 32]
    w["fb1c"] = d["fb1"].reshape(32, 1)
    w["fW2"] = d["fW2"]                               # [32, 32]
    w["fb2c"] = d["fb2"].reshape(32, 1)
    w["qWs"] = d["qW"] * 0.25                         # [32, 16] (1/sqrt(16) folded)
    w["qbsc"] = (d["qb"] * 0.25).reshape(16, 1)
    w["kW"] = d["kW"]
    w["kbc"] = d["kb"].reshape(16, 1)
    w["vW"] = d["vW"]
    w["vbc"] = d["vb"].reshape(16, 1)
    w["eW1d"] = d["eW1"][:6].copy()                   # [6, 64]  (defender rows)
    w["eW1c"] = d["eW1"][6:22].copy()                 # [16, 64] (ctx rows)
    w["eW1b"] = d["eW1"][22:].copy()                  # [6, 64]  (attacker rows)
    w["b1sc"] = np.concatenate([d["eb1"], d["eb1"]]).reshape(128, 1)
    W2s = np.zeros((128, 128), np.float32)
    W2s[:64, :64] = d["eW2"]
    W2s[64:, 64:] = d["eW2"]
    w["W2s"] = W2s
    w["b2sc"] = np.concatenate([d["eb2"], d["eb2"]]).reshape(128, 1)
    # W3pair[:, h, :]: L3 stationary for pair-half h, M=128 at base 0 so the
    # two halves accumulate into one [128,512] PSUM tile (f32r requires
    # base-0 outputs). Half h fills output rows 64h..64h+64; rest is zero.
    W3pair = np.zeros((128, 2, 128), np.float32)
    for h in range(2):
        W3pair[0:64, h, 64 * h:64 * h + 32] = d["eW3"]
        W3pair[64:128, h, 64 * h + 32:64 * h + 64] = d["eW3"]
    w["W3pair"] = W3pair
    w["b3sc2"] = np.concatenate([d["eb3"]] * 4).reshape(128, 1)
    # W4pair[:, j, :]: chunk-pair j's [128, 32] stationary for the fused L4
    # over two chunks (2c, 2c+1) stacked in partitions. Chunk c's eW4 copies
    # land in columns c (rows 0:32, d=dp points) and c+16 (rows 32:64,
    # d=dp+64 points), so PSUM row p = c + 16 r maps to defenders d=4p..4p+3.
    W4pair = np.zeros((128, NCHUNK // 2, 32), np.float32)
    for j in range(NCHUNK // 2):
        c0, c1 = 2 * j, 2 * j + 1
        W4pair[0:32, j, c0] = d["eW4"][:, 0]
        W4pair[32:64, j, c0 + 16] = d["eW4"][:, 0]
        W4pair[64:96, j, c1] = d["eW4"][:, 0]
        W4pair[96:128, j, c1 + 16] = d["eW4"][:, 0]
    w["W4pair"] = W4pair
    w["b4c"] = np.full((32, 1), float(d["eb4"][0]), np.float32)
    out = {k: np.ascontiguousarray(v, np.float32) for k, v in w.items()}
    if EVICT16:
        bf = mybir.dt.np(BF16)
        for k in ("W2s", "W3pair", "W4pair"):
            out[k] = np.ascontiguousarray(out[k].astype(bf))
    return out


def build_nc(repeat=None):
    global REPEAT
    if repeat is not None:
        REPEAT = repeat
    nc = bacc.Bacc()

    def_in = nc.dram_tensor("def_feat", [BPC, ND, 6], F32, kind="ExternalInput")
    att_in = nc.dram_tensor("att_feat", [BPC, NA, 6], F32, kind="ExternalInput")
    wshapes = {
        "fW1": [6, 32], "fb1c": [32, 1], "fW2": [32, 32], "fb2c": [32, 1],
        "qWs": [32, 16], "qbsc": [16, 1], "kW": [32, 16], "kbc": [16, 1],
        "vW": [32, 16], "vbc": [16, 1], "eW1d": [6, 64], "eW1c": [16, 64],
        "eW1b": [6, 64],
        "b1sc": [128, 1], "W2s": [128, 128], "b2sc": [128, 1],
        "W3pair": [128, 2, 128], "b3sc2": [128, 1],
        "W4pair": [128, NCHUNK // 2, 32], "b4c": [32, 1],
    }
    mm_weights = {"W2s", "W3pair", "W4pair"}
    wdram = {k: nc.dram_tensor(k, s, FMM if k in mm_weights else F32,
                               kind="ExternalInput")
             for k, s in wshapes.items()}
    out_dram = nc.dram_tensor("out", [BPC, ND, NA], F32, kind="ExternalOutput")

    with tile.TileContext(nc) as tc:
        with (
            tc.tile_pool(name="const", bufs=1) as const,
            tc.tile_pool(name="wide", bufs=1) as wide,
            tc.tile_pool(name="uv", bufs=1) as uv,
            tc.tile_pool(name="pre", bufs=3) as pre,
            tc.tile_pool(name="h1p", bufs=3) as h1p,
            tc.tile_pool(name="h2p", bufs=3) as h2p,
            tc.tile_pool(name="h3p", bufs=3) as h3p,
            tc.tile_pool(name="outp", bufs=2) as outp,
            tc.tile_pool(name="preps", bufs=2, space="PSUM") as preps,
            tc.tile_pool(name="ps2p", bufs=2, space="PSUM") as ps2p,
            tc.tile_pool(name="ps3p", bufs=2, space="PSUM") as ps3p,
            tc.tile_pool(name="outps", bufs=2, space="PSUM") as outps,
        ):
            wsb = {}
            for k, s in wshapes.items():
                wsb[k] = const.tile(s, FMM if k in mm_weights else F32,
                                    tag=k, name=k)
                nc.sync.dma_start(out=wsb[k][:], in_=wdram[k][:])
            identity = const.tile([128, 128], F32, tag="identity")
            make_identity(nc, identity[:])
            def_all = const.tile([128, BPC, 6], F32, tag="def_all")
            att_all = const.tile([128, BPC, 6], F32, tag="att_all")
            nc.sync.dma_start(out=def_all[:],
                              in_=def_in.rearrange("b d f -> d b f"))
            nc.sync.dma_start(out=att_all[:],
                              in_=att_in.rearrange("b a f -> a b f"))
            # feature-major inputs via PE transpose: [feat, batch, pos]
            def_T_all = const.tile([6, BPC, 128], F32, tag="def_T_all")
            att_T_all = const.tile([6, BPC, 128], F32, tag="att_T_all")
            for b in range(BPC):
                ps_t1 = preps.tile([6, 128], F32, tag="preps", name="ps_t1")
                nc.tensor.transpose(ps_t1[:], def_all[:, b, :], identity[:])
                nc.vector.tensor_copy(def_T_all[:, b, :], ps_t1[:])
                ps_t2 = preps.tile([6, 128], F32, tag="preps", name="ps_t2")
                nc.tensor.transpose(ps_t2[:], att_all[:, b, :], identity[:])
                nc.vector.tensor_copy(att_T_all[:, b, :], ps_t2[:])

            for rep in range(REPEAT):
                # ---- batched QKV net over all 8 batches (N=512 x2 wide) ----
                defT_flat = def_T_all[:].rearrange("f b d -> f (b d)")
                h1a_all = wide.tile([32, BPC * 128], F32, tag="h1a_all")
                h2a_all = wide.tile([32, BPC * 128], F32, tag="h2a_all")
                qT_all = wide.tile([16, BPC * 128], F32, tag="qT_all")
                kT_all = wide.tile([16, BPC * 128], F32, tag="kT_all")
                for dst, mout, lhs, rhs, bias, func in (
                    (h1a_all, 32, "fW1", defT_flat, "fb1c", AF.Relu),
                    (h2a_all, 32, "fW2", h1a_all[:], "fb2c", AF.Relu),
                    (qT_all, 16, "qWs", h2a_all[:], "qbsc", AF.Identity),
                    (kT_all, 16, "kW", h2a_all[:], "kbc", AF.Identity),
                ):
                    for s in range(BPC * 128 // 512):
                        sl = slice(512 * s, 512 * (s + 1))
                        ps_ = ps3p.tile([mout, 512], F32, tag="ps3",
                                        name="ps_wide")
                        nc.tensor.matmul(ps_[:], wsb[lhs][:], rhs[:, sl],
                                         start=True, stop=True)
                        nc.scalar.activation(dst[:, sl], ps_[:], func,
                                             bias=wsb[bias][:])

                # ---- per-batch attention preambles (independent chains) ----
                U2s, V2s = [], []
                for b in range(BPC):
                    bs = slice(128 * b, 128 * (b + 1))
                    def_T = def_T_all[:, b, :]
                    att_T = att_T_all[:, b, :]

                    ps_v = preps.tile([128, 16], F32, tag="preps", name="ps_v")
                    nc.tensor.matmul(ps_v[:], h2a_all[:, bs], wsb["vW"][:],
                                     start=True, stop=True)
                    v0 = pre.tile([128, 16], F32, tag="v0")
                    nc.vector.tensor_copy(v0[:], ps_v[:])

                    ps_sc = preps.tile([128, 128], F32, tag="preps",
                                       name="ps_sc")
                    nc.tensor.matmul(ps_sc[:], qT_all[:, bs], kT_all[:, bs],
                                     start=True, stop=True)
                    rmax_neg = pre.tile([128, 1], F32, tag="rmax_neg")
                    nc.vector.tensor_reduce(rmax_neg[:], ps_sc[:],
                                            axis=mybir.AxisListType.X,
                                            op=ALU.max, negate=True)
                    e_sb = pre.tile([128, 128], F32, tag="e_sb")
                    rsum = pre.tile([128, 1], F32, tag="rsum")
                    nc.scalar.activation(e_sb[:], ps_sc[:], AF.Exp,
                                         bias=rmax_neg[:], accum_out=rsum[:])
                    rinv = pre.tile([128, 1], F32, tag="rinv")
                    nc.vector.reciprocal(rinv[:], rsum[:])
                    attn = pre.tile([128, 128], F32, tag="attn")
                    nc.vector.tensor_scalar_mul(attn[:], e_sb[:], rinv[:])

                    ps_at = preps.tile([128, 128], F32, tag="preps",
                                       name="ps_at")
                    nc.tensor.transpose(ps_at[:], attn[:], identity[:])
                    attn_T = pre.tile([128, 128], F32, tag="attn_T")
                    nc.vector.tensor_copy(attn_T[:], ps_at[:])

                    ps_ctx = preps.tile([16, 128], F32, tag="preps",
                                        name="ps_ctx")
                    nc.tensor.matmul(ps_ctx[:], v0[:], attn_T[:],
                                     start=True, stop=True)
                    ctx_T = pre.tile([16, 128], F32, tag="ctx_T")
                    nc.scalar.activation(ctx_T[:], ps_ctx[:], AF.Identity,
                                         bias=wsb["vbc"][:])

                    # U2[p, dp] = (eW1d.T @ def_T + eW1c.T @ ctx_T)[:, dp(+64)]
                    ps_u2 = preps.tile([128, 64], F32, tag="preps",
                                       name="ps_u2")
                    nc.tensor.matmul(ps_u2[0:64, :], wsb["eW1d"][:],
                                     def_T[:, 0:64], start=True, stop=False)
                    nc.tensor.matmul(ps_u2[0:64, :], wsb["eW1c"][:],
                                     ctx_T[:, 0:64], start=False, stop=True)
                    nc.tensor.matmul(ps_u2[64:128, :], wsb["eW1d"][:],
                                     def_T[:, 64:128], start=True, stop=False)
                    nc.tensor.matmul(ps_u2[64:128, :], wsb["eW1c"][:],
                                     ctx_T[:, 64:128], start=False, stop=True)
                    U2 = uv.tile([128, 64], F32, tag=f"U2_{b}", name="U2")
                    nc.vector.tensor_copy(U2[:], ps_u2[:])

                    ps_v2 = preps.tile([128, 128], F32, tag="preps",
                                       name="ps_v2")
                    nc.tensor.matmul(ps_v2[0:64, :], wsb["eW1b"][:], att_T,
                                     start=True, stop=True)
                    nc.tensor.matmul(ps_v2[64:128, :], wsb["eW1b"][:], att_T,
                                     start=True, stop=True)
                    V2 = uv.tile([128, 128], BF16 if EVICT16 else F32,
                                 tag=f"V2_{b}", name="V2")
                    nc.scalar.activation(V2[:], ps_v2[:], AF.Identity,
                                         bias=wsb["b1sc"][:])
                    U2s.append(U2)
                    V2s.append(V2)

                # ---- main loops ----
                for b in range(BPC):
                    _main_loop(nc, wsb, outp, outps, h1p, h2p, h3p,
                               ps2p, ps3p, U2s[b], V2s[b], out_dram, b)
    nc.finalize()
    return nc


def _main_loop(nc, wsb, outp, outps, h1p, h2p, h3p, ps2p, ps3p,
               U2, V2, out_dram, b):
    """Pairwise MLP over one batch: 8 chunk pairs of 1024 points each."""
    out_ps = outps.tile([32, 512], F32, tag="outps", name="out_ps")
    NPAIR = NCHUNK // 2
    for j in range(NPAIR):
        ps34 = ps3p.tile([128, 512], F32, tag="ps3", name="ps34")
        for half in range(2):
            c = 2 * j + half
            h1 = h1p.tile([128, 512], FMM, tag="h1", name="h1")
            for m in range(4):
                dp = 4 * c + m
                seg = h1[:, 128 * m:128 * (m + 1)]
                eng = H1_ENGINES[m]
                if eng == "s":
                    nc.scalar.activation(seg, V2[:], AF.Relu,
                                         bias=U2[:, dp:dp + 1])
                else:
                    e_ = nc.vector if eng == "v" else nc.gpsimd
                    e_.tensor_scalar(seg, V2[:], U2[:, dp:dp + 1],
                                     0.0, ALU.add, ALU.max)
            ps2 = ps2p.tile([128, 512], F32, tag="ps2", name="ps2")
            nc.tensor.matmul(ps2[:], wsb["W2s"][:], h1[:],
                             start=True, stop=True)
            h2 = h2p.tile([128, 512], FMM, tag="h2", name="h2")
            nc.scalar.activation(h2[:], ps2[:], AF.Relu, bias=wsb["b2sc"][:])
            # two chunks' L3 outputs stack into one [128,512] psum
            # (accumulating M=128 matmuls with half-zero weights)
            nc.tensor.matmul(ps34[:], wsb["W3pair"][:, half, :], h2[:],
                             start=(half == 0), stop=(half == 1))
        h34 = h3p.tile([128, 512], FMM, tag="h3", name="h34")
        nc.vector.tensor_scalar(h34[:], ps34[:], wsb["b3sc2"][:],
                                0.0, ALU.add, ALU.max)
        nc.tensor.matmul(out_ps[:], wsb["W4pair"][:, j, :], h34[:],
                         start=(j == 0), stop=(j == NPAIR - 1))

    out_sb = outp.tile([32, 512], F32, tag="out_sb", name="out_sb")
    nc.scalar.activation(out_sb[:], out_ps[:], AF.Identity, bias=wsb["b4c"][:])
    # row p holds defenders d = 4p..4p+3 -> contiguous store
    nc.sync.dma_start(
        out=out_dram[b].rearrange("(p m) a -> p (m a)", p=32),
        in_=out_sb[:])


_NC_CACHE = {}


def kernel(defender_features, attacker_features, **weights):
    """Full-input entry point: shards batch across 8 cores, returns [64,128,128]."""
    from concourse import bass2jax

    if "nc" not in _NC_CACHE:
        _NC_CACHE["nc"] = build_nc()
    nc = _NC_CACHE["nc"]

    w = _weight_tensors(weights)
    df = np.ascontiguousarray(defender_features, np.float32)
    af = np.ascontiguousarray(attacker_features, np.float32)
    in_maps = []
    for i in range(N_CORES):
        m = {"def_feat": df[i * BPC:(i + 1) * BPC],
             "att_feat": af[i * BPC:(i + 1) * BPC]}
        m.update(w)
        in_maps.append(m)

    results = bass2jax.run_bass_via_pjrt(nc, in_maps, n_cores=N_CORES)
    return np.concatenate([results[i]["out"] for i in range(N_CORES)], axis=0)


if __name__ == "__main__":
    build_nc()
    print("build ok")
